# revision 39
# baseline (speedup 1.0000x reference)
"""Trainium2 Bass kernel for AdaptiveProjection (dense MoE routing).

Computes: out[b,s,:] = sum_e softmax(x@gate_w.T + gate_b)[b,s,e] * (x[b,s] @ W_e.T)

Sharding: data-parallel over (B*S) across 8 NeuronCores; weights replicated.
Per core: 2048 tokens, full 4x1024x1024 expert weights resident in SBUF (bf16).
"""

import numpy as np
import ml_dtypes

B, S, D, O, E = 4, 4096, 1024, 1024, 4
N_CORES = 8
T = (B * S) // N_CORES  # 2048 tokens per core
KC = D // 128           # 8 contraction chunks of 128
NT = T // 128           # 16 token tiles per core
NH = O // 512           # 2 output halves
BLK = 512               # gate-logit token block

_CACHE = {}


def _build_graph():
    import concourse.mybir as mybir
    from concourse import bacc
    from concourse.bass import ts, ds
    from concourse.tile import TileContext

    f32 = mybir.dt.float32
    bf16 = mybir.dt.bfloat16
    nc = bacc.Bacc(None, target_bir_lowering=False)

    xt_d = nc.declare_dram_parameter("xt", [KC, 128, T], bf16, isOutput=False)
    wt_d = nc.declare_dram_parameter("wt", [NH, KC, 128, E, 512], bf16, isOutput=False)
    gwt_d = nc.declare_dram_parameter("gwt", [KC, 128, E], bf16, isOutput=False)
    gb_d = nc.declare_dram_parameter("gb", [E, 1], f32, isOutput=False)
    id_d = nc.declare_dram_parameter("ident", [E, E], bf16, isOutput=False)
    out_d = nc.declare_dram_parameter("out", [T, O], f32, isOutput=True)

    with TileContext(nc) as tc:
        with (
            tc.tile_pool(name="persist", bufs=1) as pp,
            tc.tile_pool(name="gate_sm", bufs=4) as gp,
            tc.tile_pool(name="acc", bufs=8) as ap,
        ):
            # --- persistent SBUF tensors ---
            xt_sb = pp.tile([128, KC, T], bf16, tag="xt")
            w_sb = pp.tile([128, NH, KC, E, 512], bf16, tag="w")
            gw_sb = pp.tile([128, KC, E], bf16, tag="gw")
            gb_sb = pp.tile([E, 1], f32, tag="gb")
            id_sb = pp.tile([E, E], bf16, tag="ident")
            exp_sb = pp.tile([E, T], bf16, tag="exprow")
            gates_sb = pp.tile([128, NT * E], f32, tag="gates")

            # --- loads ---
            # All loads go on the sync ring ONLY: the scalar(ACT) engine must
            # stay free for the exp/epilogue work — DMA dispatches block on
            # queue backpressure and would stall its instruction FIFO.
            # Tiny tensors go via gpsimd SWDGE (separate queue).
            scratch = pp.tile([128, 512], bf16, tag="scratch")
            nc.vector.memset(scratch[:, :], 0)
            # gb/id ride the scalar ring (2 tiny dispatches, well before the
            # exps need the engine); keeps gpsimd entirely idle
            nc.scalar.dma_start(out=gb_sb[:, :], in_=gb_d[:, :])
            nc.scalar.dma_start(out=id_sb[:, :], in_=id_d[:, :])
            # gw unblocks the first gate matmul — put it at the head of the
            # sync ring (8KB, negligible delay to the xt stream)
            nc.sync.dma_start(
                out=gw_sb[:, :, :], in_=gwt_d.rearrange("k p e -> p k e")
            )
            # xt first (it gates the gate prologue), then W half-0 (h=0
            # matmuls trail the stream), then W half-1 (hidden under h=0).
            for k in range(KC):
                nc.sync.dma_start(out=xt_sb[:, k, :], in_=xt_d[k])
            for k in range(KC):
                nc.sync.dma_start(out=w_sb[:, 0, k, :, :], in_=wt_d[0, k])
            for k in range(KC):
                nc.sync.dma_start(out=w_sb[:, 1, k, :, :], in_=wt_d[1, k])

            # --- gate prologue ---
            # PE warm-up: no-dep matmuls on a scratch tile fill the DMA
            # lead-in and release the HAM clock throttle before real work.
            with (
                tc.tile_pool(name="psum_w", bufs=1, space="PSUM") as pwp,
                tc.tile_pool(name="psum_g", bufs=4, space="PSUM") as pgp,
                tc.tile_pool(name="psum_t", bufs=1, space="PSUM") as ptp,
            ):
                warm_ps = pwp.tile([128, 512], f32, tag="warm")
                for _ in range(12):
                    nc.tensor.matmul(
                        warm_ps[:, :],
                        scratch[:, 0:128],
                        scratch[:, :],
                        start=True,
                        stop=True,
                        skip_group_check=True,
                    )
                # logits in row layout [E, tokens]: stationary = gwT [128, E];
                # k outer so each xt chunk is consumed as soon as it lands.
                glogs = [
                    pgp.tile([E, BLK], f32, tag="glog", name=f"glog{b}")
                    for b in range(T // BLK)
                ]
                for k in range(KC):
                    for b in range(T // BLK):
                        nc.tensor.matmul(
                            glogs[b][:, :],
                            gw_sb[:, k, :],
                            xt_sb[:, k, ts(b, BLK)],
                            start=(k == 0),
                            stop=(k == KC - 1),
                        )
                # exp (ACT) and transpose (PE) interleaved at 256-token grain
                # so the transposes start as soon as the first exp chunk lands
                expT = ptp.tile([128, NT * E], bf16, tag="expT")
                for c in range(T // 256):
                    b, off = c // 2, (c % 2) * 256
                    nc.scalar.activation(
                        exp_sb[:, b * BLK + off : b * BLK + off + 256],
                        glogs[b][:, off : off + 256],
                        mybir.ActivationFunctionType.Exp,
                        bias=gb_sb[:, 0:1],
                        scale=1.0,
                    )
                    for t in (2 * c, 2 * c + 1):
                        nc.tensor.transpose(
                            expT[:, ts(t, E)],
                            exp_sb[:, ts(t, 128)],
                            id_sb[:, :],
                        )
                denom = gp.tile([128, NT], f32, tag="denom")
                recip = gp.tile([128, NT], f32, tag="recip")
                expT3 = expT[:, :].rearrange("p (t e) -> p t e", e=E)
                nc.vector.reduce_sum(
                    denom[:, :], expT3, axis=mybir.AxisListType.X
                )
                nc.vector.reciprocal(recip[:, :], denom[:, :])
                nc.vector.tensor_mul(
                    gates_sb[:, :].rearrange("p (t e) -> p t e", e=E),
                    expT3,
                    recip[:, :, None].broadcast_to([128, NT, E]),
                )

            # --- main loop: expert matmuls + gated combine ---
            with tc.tile_pool(name="psum_e", bufs=8, space="PSUM") as pep:
                for h in range(NH):
                    for t in range(NT):
                        psums = [
                            pep.tile([128, 512], f32, tag="ep", name=f"ep{t}_{h}_{e}")
                            for e in range(E)
                        ]
                        for k in range(KC):
                            lhs = xt_sb[:, k, ts(t, 128)]
                            for e in range(E):
                                nc.tensor.matmul(
                                    psums[e][:, :],
                                    lhs,
                                    w_sb[:, h, k, e, :],
                                    start=(k == 0),
                                    stop=(k == KC - 1),
                                )
                        # gated combine: two independent halves
                        #   a = p0*g0 (ACT); a = p1*g1 + a (DVE)
                        #   b = p2*g2 (ACT); b = p3*g3 + b (DVE)
                        #   acc = a + b (DVE, SBUF-only 2x mode)
                        ha = ap.tile([128, 512], f32, tag="ha")
                        hb = ap.tile([128, 512], f32, tag="hb")
                        acc = ap.tile([128, 512], f32, tag="acc")
                        for half, (e0, e1) in ((ha, (0, 1)), (hb, (2, 3))):
                            nc.scalar.activation(
                                half[:, :],
                                psums[e0][:, :],
                                mybir.ActivationFunctionType.Copy,
                                bias=0.0,
                                scale=gates_sb[:, t * E + e0 : t * E + e0 + 1],
                            )
                            nc.vector.scalar_tensor_tensor(
                                half[:, :],
                                psums[e1][:, :],
                                gates_sb[:, t * E + e1 : t * E + e1 + 1],
                                half[:, :],
                                op0=mybir.AluOpType.mult,
                                op1=mybir.AluOpType.add,
                            )
                        nc.vector.tensor_add(acc[:, :], ha[:, :], hb[:, :])
                        nc.scalar.dma_start(
                            out=out_d[ts(t, 128), ds(512 * h, 512)], in_=acc[:, :]
                        )
    nc.compile()
    return nc


def _prep_inputs(x, W_experts, gate_w, gate_b):
    bf16 = ml_dtypes.bfloat16
    x_flat = np.asarray(x, dtype=np.float32).reshape(B * S, D)
    # weights (replicated): wt[h,k,p,e,o'] = W[e,512h+o',128k+p]
    wt = (
        np.ascontiguousarray(
            np.asarray(W_experts, dtype=np.float32)
            .reshape(E, NH, 512, D)
            .transpose(1, 3, 0, 2)  # -> [NH, D, E, 512]
        )
        .reshape(NH, KC, 128, E, 512)
        .astype(bf16)
    )
    gwt = (
        np.ascontiguousarray(np.asarray(gate_w, dtype=np.float32).T)
        .reshape(KC, 128, E)
        .astype(bf16)
    )
    gb = np.asarray(gate_b, dtype=np.float32).reshape(E, 1)
    ident = np.eye(E, dtype=np.float32).astype(bf16)

    in_maps = []
    for i in range(N_CORES):
        shard = x_flat[i * T : (i + 1) * T]  # [T, D]
        xt = (
            np.ascontiguousarray(shard.T).reshape(KC, 128, T).astype(bf16)
        )
        in_maps.append(
            {"xt": xt, "wt": wt, "gwt": gwt, "gb": gb, "ident": ident}
        )
    return in_maps


def _run(inputs, trace=False):
    from concourse.bass_utils import run_bass_kernel_spmd

    if "nc" not in _CACHE:
        _CACHE["nc"] = _build_graph()
    nc = _CACHE["nc"]
    in_maps = _prep_inputs(**inputs)
    res = run_bass_kernel_spmd(
        nc, in_maps, core_ids=list(range(N_CORES)), trace=trace
    )
    shards = [np.asarray(res.results[i]["out"]) for i in range(N_CORES)]
    out = np.concatenate(shards, axis=0).reshape(B, S, O).astype(np.float32)
    return out, res


def kernel(x, W_experts, gate_w, gate_b):
    out, _ = _run(
        {"x": x, "W_experts": W_experts, "gate_w": gate_w, "gate_b": gate_b}
    )
    return out


# revision 40
# speedup vs baseline: 1.0200x; 1.0200x over previous
"""Trainium2 Bass kernel for AdaptiveProjection (dense MoE routing).

Computes: out[b,s,:] = sum_e softmax(x@gate_w.T + gate_b)[b,s,e] * (x[b,s] @ W_e.T)

Sharding: data-parallel over (B*S) across 8 NeuronCores; weights replicated.
Per core: 2048 tokens, full 4x1024x1024 expert weights resident in SBUF (bf16).
"""

import numpy as np
import ml_dtypes

B, S, D, O, E = 4, 4096, 1024, 1024, 4
N_CORES = 8
T = (B * S) // N_CORES  # 2048 tokens per core
KC = D // 128           # 8 contraction chunks of 128
NT = T // 128           # 16 token tiles per core
NH = O // 512           # 2 output halves
BLK = 512               # gate-logit token block

_CACHE = {}


def _build_graph():
    import concourse.mybir as mybir
    from concourse import bacc
    from concourse.bass import ts, ds
    from concourse.tile import TileContext

    f32 = mybir.dt.float32
    bf16 = mybir.dt.bfloat16
    nc = bacc.Bacc(None, target_bir_lowering=False)

    xt_d = nc.declare_dram_parameter("xt", [KC, 128, T], bf16, isOutput=False)
    wt_d = nc.declare_dram_parameter("wt", [NH, KC, 128, E, 512], bf16, isOutput=False)
    gwt_d = nc.declare_dram_parameter("gwt", [KC, 128, E], bf16, isOutput=False)
    gb_d = nc.declare_dram_parameter("gb", [E, 1], f32, isOutput=False)
    id_d = nc.declare_dram_parameter("ident", [E, E], bf16, isOutput=False)
    out_d = nc.declare_dram_parameter("out", [T, O], f32, isOutput=True)

    with TileContext(nc) as tc:
        with (
            tc.tile_pool(name="persist", bufs=1) as pp,
            tc.tile_pool(name="gate_sm", bufs=4) as gp,
            tc.tile_pool(name="acc", bufs=8) as ap,
        ):
            # --- persistent SBUF tensors ---
            xt_sb = pp.tile([128, KC, T], bf16, tag="xt")
            w_sb = pp.tile([128, NH, KC, E, 512], bf16, tag="w")
            gw_sb = pp.tile([128, KC, E], bf16, tag="gw")
            gb_sb = pp.tile([E, 1], f32, tag="gb")
            id_sb = pp.tile([E, E], bf16, tag="ident")
            exp_sb = pp.tile([E, T], bf16, tag="exprow")
            gates_sb = pp.tile([128, NT * E], f32, tag="gates")

            # --- loads ---
            # All loads go on the sync ring ONLY: the scalar(ACT) engine must
            # stay free for the exp/epilogue work — DMA dispatches block on
            # queue backpressure and would stall its instruction FIFO.
            # Tiny tensors go via gpsimd SWDGE (separate queue).
            scratch = pp.tile([128, 512], bf16, tag="scratch")
            nc.vector.memset(scratch[:, :], 0)
            # gb/id ride the scalar ring (2 tiny dispatches, well before the
            # exps need the engine); keeps gpsimd entirely idle
            nc.scalar.dma_start(out=gb_sb[:, :], in_=gb_d[:, :])
            nc.scalar.dma_start(out=id_sb[:, :], in_=id_d[:, :])
            # gw unblocks the first gate matmul — put it at the head of the
            # sync ring (8KB, negligible delay to the xt stream)
            nc.sync.dma_start(
                out=gw_sb[:, :, :], in_=gwt_d.rearrange("k p e -> p k e")
            )
            # xt first (it gates the gate prologue), then W half-0 (h=0
            # matmuls trail the stream), then W half-1 (hidden under h=0).
            for k in range(KC):
                nc.sync.dma_start(out=xt_sb[:, k, :], in_=xt_d[k])
            for k in range(KC):
                nc.sync.dma_start(out=w_sb[:, 0, k, :, :], in_=wt_d[0, k])
            for k in range(KC):
                nc.sync.dma_start(out=w_sb[:, 1, k, :, :], in_=wt_d[1, k])

            # --- gate prologue ---
            # PE warm-up: no-dep matmuls on a scratch tile fill the DMA
            # lead-in and release the HAM clock throttle before real work.
            with (
                tc.tile_pool(name="psum_w", bufs=1, space="PSUM") as pwp,
                tc.tile_pool(name="psum_g", bufs=4, space="PSUM") as pgp,
                tc.tile_pool(name="psum_t", bufs=1, space="PSUM") as ptp,
            ):
                warm_ps = pwp.tile([128, 512], f32, tag="warm")
                for _ in range(12):
                    nc.tensor.matmul(
                        warm_ps[:, :],
                        scratch[:, 0:128],
                        scratch[:, :],
                        start=True,
                        stop=True,
                        skip_group_check=True,
                    )
                # logits in row layout [E, tokens]: stationary = gwT [128, E];
                # k outer so each xt chunk is consumed as soon as it lands.
                glogs = [
                    pgp.tile([E, BLK], f32, tag="glog", name=f"glog{b}")
                    for b in range(T // BLK)
                ]
                for k in range(KC):
                    for b in range(T // BLK):
                        nc.tensor.matmul(
                            glogs[b][:, :],
                            gw_sb[:, k, :],
                            xt_sb[:, k, ts(b, BLK)],
                            start=(k == 0),
                            stop=(k == KC - 1),
                        )
                # exp(logits + gate_b) on ACT; bias is per-partition [E,1]
                for b in range(T // BLK):
                    nc.scalar.activation(
                        exp_sb[:, ts(b, BLK)],
                        glogs[b][:, :],
                        mybir.ActivationFunctionType.Exp,
                        bias=gb_sb[:, 0:1],
                        scale=1.0,
                    )
                # transpose exp rows -> [128, E] per token tile
                expT = ptp.tile([128, NT * E], bf16, tag="expT")
                for t in range(NT):
                    nc.tensor.transpose(
                        expT[:, ts(t, E)],
                        exp_sb[:, ts(t, 128)],
                        id_sb[:, :],
                    )
                denom = gp.tile([128, NT], f32, tag="denom")
                recip = gp.tile([128, NT], f32, tag="recip")
                expT3 = expT[:, :].rearrange("p (t e) -> p t e", e=E)
                nc.vector.reduce_sum(
                    denom[:, :], expT3, axis=mybir.AxisListType.X
                )
                nc.vector.reciprocal(recip[:, :], denom[:, :])
                nc.vector.tensor_mul(
                    gates_sb[:, :].rearrange("p (t e) -> p t e", e=E),
                    expT3,
                    recip[:, :, None].broadcast_to([128, NT, E]),
                )

            # --- main loop: expert matmuls + gated combine ---
            with tc.tile_pool(name="psum_e", bufs=8, space="PSUM") as pep:
                for h in range(NH):
                    for t in range(NT):
                        psums = [
                            pep.tile([128, 512], f32, tag="ep", name=f"ep{t}_{h}_{e}")
                            for e in range(E)
                        ]
                        for k in range(KC):
                            lhs = xt_sb[:, k, ts(t, 128)]
                            for e in range(E):
                                nc.tensor.matmul(
                                    psums[e][:, :],
                                    lhs,
                                    w_sb[:, h, k, e, :],
                                    start=(k == 0),
                                    stop=(k == KC - 1),
                                )
                        # gated combine: two independent halves
                        #   a = p0*g0 (ACT); a = p1*g1 + a (DVE)
                        #   b = p2*g2 (ACT); b = p3*g3 + b (DVE)
                        #   acc = a + b (DVE, SBUF-only 2x mode)
                        ha = ap.tile([128, 512], f32, tag="ha")
                        hb = ap.tile([128, 512], f32, tag="hb")
                        acc = ap.tile([128, 512], f32, tag="acc")
                        for half, (e0, e1) in ((ha, (0, 1)), (hb, (2, 3))):
                            nc.scalar.activation(
                                half[:, :],
                                psums[e0][:, :],
                                mybir.ActivationFunctionType.Copy,
                                bias=0.0,
                                scale=gates_sb[:, t * E + e0 : t * E + e0 + 1],
                            )
                            nc.vector.scalar_tensor_tensor(
                                half[:, :],
                                psums[e1][:, :],
                                gates_sb[:, t * E + e1 : t * E + e1 + 1],
                                half[:, :],
                                op0=mybir.AluOpType.mult,
                                op1=mybir.AluOpType.add,
                            )
                        nc.vector.tensor_add(acc[:, :], ha[:, :], hb[:, :])
                        nc.scalar.dma_start(
                            out=out_d[ts(t, 128), ds(512 * h, 512)], in_=acc[:, :]
                        )
    nc.compile()
    return nc


def _prep_inputs(x, W_experts, gate_w, gate_b):
    bf16 = ml_dtypes.bfloat16
    x_flat = np.asarray(x, dtype=np.float32).reshape(B * S, D)
    # weights (replicated): wt[h,k,p,e,o'] = W[e,512h+o',128k+p]
    wt = (
        np.ascontiguousarray(
            np.asarray(W_experts, dtype=np.float32)
            .reshape(E, NH, 512, D)
            .transpose(1, 3, 0, 2)  # -> [NH, D, E, 512]
        )
        .reshape(NH, KC, 128, E, 512)
        .astype(bf16)
    )
    gwt = (
        np.ascontiguousarray(np.asarray(gate_w, dtype=np.float32).T)
        .reshape(KC, 128, E)
        .astype(bf16)
    )
    gb = np.asarray(gate_b, dtype=np.float32).reshape(E, 1)
    ident = np.eye(E, dtype=np.float32).astype(bf16)

    in_maps = []
    for i in range(N_CORES):
        shard = x_flat[i * T : (i + 1) * T]  # [T, D]
        xt = (
            np.ascontiguousarray(shard.T).reshape(KC, 128, T).astype(bf16)
        )
        in_maps.append(
            {"xt": xt, "wt": wt, "gwt": gwt, "gb": gb, "ident": ident}
        )
    return in_maps


def _run(inputs, trace=False):
    from concourse.bass_utils import run_bass_kernel_spmd

    if "nc" not in _CACHE:
        _CACHE["nc"] = _build_graph()
    nc = _CACHE["nc"]
    in_maps = _prep_inputs(**inputs)
    res = run_bass_kernel_spmd(
        nc, in_maps, core_ids=list(range(N_CORES)), trace=trace
    )
    shards = [np.asarray(res.results[i]["out"]) for i in range(N_CORES)]
    out = np.concatenate(shards, axis=0).reshape(B, S, O).astype(np.float32)
    return out, res


def kernel(x, W_experts, gate_w, gate_b):
    out, _ = _run(
        {"x": x, "W_experts": W_experts, "gate_w": gate_w, "gate_b": gate_b}
    )
    return out
